# revision 18
# baseline (speedup 1.0000x reference)
"""Trainium2 Bass kernel for nn_GAU_66503273612026 (GAU with diagonal-only attention).

Math (per batch element b, x_b: [T=2048, D=1024]):
    hidden = silu(x_b @ W_hidden + b_hidden)        # [T, 2*TFO]
    v, gate = split(hidden)                          # [T, TFO] each
    z = silu(x_b @ W_qk + b_qk)                      # [T, TFO]
    q = (z*gamma0 + beta0) / sqrt(TFO); k = z*gamma1 + beta1
    sim = q @ k^T                                    # [T, T] (tiny values)
    d_i = exp(sim_ii) / sum_j exp(sim_ij)            # diagonal of softmax only
    V = d[:,None] * v * gate
    out_b = (V @ W_out + b_out)^T                    # [NODES, T]
Final output: stack over b -> [B, 1, NODES, T].

Sharding: data-parallel over B: batch element b -> NeuronCore b (8 cores).

Engine plan (cost-model-driven):
  PE   : fp8 DoubleRow for the three projections + MM4; bf16 for the 16
         diagonal sim blocks + sbar matvecs.  ~62 us -> the wall.
  ACT  : the three silus only, batched over 2 PSUM banks (ap=1024), bf16 out.
  DVE  : q/k derives in bf16 (TensorScalarPtr 4x mode), post-MM4 d-multiply
         and bias-add (f32), diag-mask accum, stats.
  Pool : V8 = sv*8*sg -> fp8 (scalar_tensor_tensor, ACT/DVE offload).
  The softmax never needs exp: |sim| << 1, so
    d_i = (1 + sim_ii) / (T + sum_j sim_ij)   to ~1e-8 relative, and the
  row sum collapses via linearity to q_i . Kbar with Kbar = sum_j k_j.
"""

import numpy as np
from contextlib import ExitStack

B, T, D, TFO, NODES = 8, 2048, 1024, 1024, 1024
P = 128
FT = 512            # free-dim tile (one PSUM bank of f32)
TW = 1024           # token window (2 PSUM banks)
NW = T // TW        # 2 windows
NT = T // FT        # 4 token tiles
DC = D // P         # 8 contraction chunks over D
OC = TFO // P       # 8 feature chunks over TFO
NC_ = NODES // P    # 8 output row chunks
IC = T // P         # 16 row chunks for attention stats

_compiled_nc = None


def _build():
    import concourse.bass as bass
    import concourse.tile as tile
    from concourse import bacc, mybir
    from concourse.bass import ts
    from concourse.masks import make_identity

    f32 = mybir.dt.float32
    bf16 = mybir.dt.bfloat16
    f8 = mybir.dt.float8e4
    AF = mybir.ActivationFunctionType
    OP = mybir.AluOpType
    AX = mybir.AxisListType

    nc = bacc.Bacc("TRN2", target_bir_lowering=False, debug=False,
                   enable_asserts=False, num_devices=1)

    xT8 = nc.dram_tensor("xT8", [D, T], f8, kind="ExternalInput").ap()      # fp8(x^T)
    wqk8 = nc.dram_tensor("wqk8", [D, TFO], f8, kind="ExternalInput").ap()  # W_qk*2^8 fp8
    wh8 = nc.dram_tensor("wh8", [D, 2 * TFO], f8, kind="ExternalInput").ap()  # W_hidden*2^8
    wo8 = nc.dram_tensor("wo8", [TFO, NODES], f8, kind="ExternalInput").ap()  # W_out*2^8
    # per-chunk constant columns [P, 10, 8]; plane i:
    # 0 bqk, 1 bv, 2 bg, 3 bo, 4 g0/32, 5 be0/32, 6 g1, 7 be1,
    # 8 g1 (kbar), 9 T*be1 (kbar).  Column c of plane i holds elems c*128..+127.
    consts = nc.dram_tensor("consts", [P, 10, OC], f32, kind="ExternalInput").ap()
    outT = nc.dram_tensor("outT", [NODES, T], f32, kind="ExternalOutput").ap()

    with tile.TileContext(nc) as tc, ExitStack() as ctx:
        persist = ctx.enter_context(tc.tile_pool(name="persist", bufs=1))
        dramp = ctx.enter_context(tc.tile_pool(name="dram", bufs=1, space="DRAM"))

        # constants: one tile, one DMA on the SWDGE queue (never queues
        # behind the HWDGE weight/activation loads)
        cst = persist.tile([P, 10, OC], f32, tag="consts")
        nc.gpsimd.dma_start(out=cst, in_=consts)
        bqk_sb, bv_sb, bg_sb, bo_sb = (cst[:, i, :] for i in range(4))
        g0_sb, be0_sb, g1_sb, be1_sb = (cst[:, i, :] for i in range(4, 8))
        g1k_sb, be1k_sb = cst[:, 8, :], cst[:, 9, :]
        ident = persist.tile([P, P], f32, tag="ident")
        make_identity(nc, ident[:])

        # fp8(x^T) resident: [p, dc, t] where d = dc*128+p
        x8_sb = persist.tile([P, DC, T], f8, tag="x8")
        # weights resident
        wqk8_sb = persist.tile([P, DC, TFO], f8, tag="wqk8")
        wh8_sb = persist.tile([P, DC, 2 * TFO], f8, tag="wh8")
        wo8_sb = persist.tile([P, OC, NODES], f8, tag="wo8")

        wqk8_r = wqk8.rearrange("(dc p) e -> p dc e", p=P)
        x8_r = xT8.rearrange("(dc p) t -> p dc t", p=P)
        wh8_r = wh8.rearrange("(dc p) e -> p dc e", p=P)
        wo8_r = wo8.rearrange("(oc p) n -> p oc n", p=P)

        # DMA order matches PE consumption: wqk oc0 slice, x window 0,
        # rest of wqk, x window 1, wh (v|g), wo.
        nc.sync.dma_start(out=wqk8_sb[:, :, ts(0, P)], in_=wqk8_r[:, :, ts(0, P)])
        nc.sync.dma_start(out=x8_sb[:, :, ts(0, FT)], in_=x8_r[:, :, ts(0, FT)])
        nc.sync.dma_start(out=x8_sb[:, :, ts(1, FT)], in_=x8_r[:, :, ts(1, FT)])
        for oc in range(1, OC):
            nc.sync.dma_start(out=wqk8_sb[:, :, ts(oc, P)],
                              in_=wqk8_r[:, :, ts(oc, P)])
        nc.sync.dma_start(out=x8_sb[:, :, ts(1, TW)], in_=x8_r[:, :, ts(1, TW)])
        for h in range(4):
            nc.sync.dma_start(out=wh8_sb[:, :, ts(h, FT)], in_=wh8_r[:, :, ts(h, FT)])
        for h in range(2):
            nc.sync.dma_start(out=wo8_sb[:, :, ts(h, FT)],
                              in_=wo8_r[:, :, ts(h, FT)])

        # bf16 q/k, feature-partitioned: [p, oc, w, h, t]
        qb_sb = persist.tile([P, OC, NW, 2, FT], bf16, tag="qb")
        kb_sb = persist.tile([P, OC, NW, 2, FT], bf16, tag="kb")
        V8_sb = persist.tile([P, OC, NW, 2, FT], f8, tag="V8")    # v*gate*d*2^11
        dbcast = persist.tile([P, NW, 2, FT], bf16, tag="dbcast")  # diag*2^11
        diag_dr = dramp.tile([T, 1], bf16, tag="diag")             # DRAM scratch
        statp = ctx.enter_context(tc.tile_pool(name="statp", bufs=4))

        # ---------------- Phase A: z = silu(x @ W_qk + b_qk) -> qb/kb (bf16)
        with ExitStack() as ab:
            zstg = ab.enter_context(tc.tile_pool(name="zstg", bufs=3))
            psA = ab.enter_context(tc.tile_pool(name="psA", bufs=2, space="PSUM"))
            psB = ab.enter_context(tc.tile_pool(name="psB", bufs=2, space="PSUM"))

            zbar = statp.tile([P, OC, NW], f32, tag="zbar")
            kbar = statp.tile([P, OC], bf16, tag="kbar")

            # p-state warmup: keep the PE continuously busy from ~0.3us so
            # it reaches full clock (3us ramp) before the first real matmul.
            # Dest is the first a_block's psum tile; its real matmuls reset
            # the bank with start=True.
            wps = psA.tile([P, 2, FT], f32, tag="zps")
            for _ in range(8):
                nc.tensor.matmul(wps[:, 0, 0:P], lhsT=ident[:], rhs=ident[:],
                                 start=True, stop=True)

            def a_block(w, oc, zps=None):
                if zps is None:
                    zps = psA.tile([P, 2, FT], f32, tag="zps")
                for h in range(2):
                    for c in range(DC // 2):
                        nc.tensor.matmul(zps[:, h, :],
                                         lhsT=wqk8_sb[:, 2 * c:2 * c + 2, ts(oc, P)],
                                         rhs=x8_sb[:, 2 * c:2 * c + 2,
                                                   ts(2 * w + h, FT)],
                                         start=(c == 0), stop=(c == DC // 2 - 1),
                                         perf_mode=mybir.MatmulPerfMode.DoubleRow)
                zt = zstg.tile([P, 2, FT], bf16, tag="zt")
                # W_qk was scaled by 2^8 into fp8; descale inside silu.
                # accum over the whole [P,2,FT] ap -> zbar[:, oc, w]
                nc.scalar.activation(out=zt[:], in_=zps[:], func=AF.Silu,
                                     bias=bqk_sb[:, oc:oc + 1], scale=2.0 ** -8,
                                     accum_out=zbar[:, oc, w:w + 1])
                # bf16 in+out, SBUF, packed -> DVE 4x mode
                nc.vector.tensor_scalar(out=qb_sb[:, oc, w], in0=zt[:],
                                        scalar1=g0_sb[:, oc:oc + 1],
                                        scalar2=be0_sb[:, oc:oc + 1],
                                        op0=OP.mult, op1=OP.add)
                nc.vector.tensor_scalar(out=kb_sb[:, oc, w], in0=zt[:],
                                        scalar1=g1_sb[:, oc:oc + 1],
                                        scalar2=be1_sb[:, oc:oc + 1],
                                        op0=OP.mult, op1=OP.add)

            def kbar_chain():
                # Kbar = gamma1 * (zbar_w0 + zbar_w1) + T*beta1, as bf16
                zsum = statp.tile([P, OC], f32, tag="zsum")
                nc.vector.tensor_tensor(out=zsum[:], in0=zbar[:, :, 0],
                                        in1=zbar[:, :, 1], op=OP.add)
                ktmp = statp.tile([P, OC], f32, tag="ktmp")
                nc.vector.tensor_tensor(out=ktmp[:], in0=zsum[:], in1=g1k_sb,
                                        op=OP.mult)
                nc.vector.tensor_tensor(out=kbar[:], in0=ktmp[:], in1=be1k_sb,
                                        op=OP.add)

            dsim_all = statp.tile([P, IC], f32, tag="dsim")
            sb_all = psB.tile([P, IC], f32, tag="sball")

            def sim_block(ic):
                w, rest = divmod(ic, IC // NW)
                h, i4 = divmod(rest, 4)
                simps = psB.tile([P, P], f32, tag="simps")
                for oc in range(OC):
                    nc.tensor.matmul(simps[:],
                                     lhsT=qb_sb[:, oc, w, h, ts(i4, P)],
                                     rhs=kb_sb[:, oc, w, h, ts(i4, P)],
                                     start=(oc == 0), stop=(oc == OC - 1))
                tmp = statp.tile([P, P], f32, tag="dtmp")
                nc.vector.scalar_tensor_tensor(
                    out=tmp[:], in0=simps[:], scalar=1.0, in1=ident[:],
                    op0=OP.mult, op1=OP.mult, accum_out=dsim_all[:, ic:ic + 1])

            def sbar_block(ic):
                w, rest = divmod(ic, IC // NW)
                h, i4 = divmod(rest, 4)
                for oc in range(OC):
                    nc.tensor.matmul(sb_all[:, ic:ic + 1],
                                     lhsT=qb_sb[:, oc, w, h, ts(i4, P)],
                                     rhs=kbar[:, oc:oc + 1],
                                     start=(oc == 0), stop=(oc == OC - 1))

            for w in range(NW):
                for oc in range(OC):
                    a_block(w, oc, zps=(wps if (w, oc) == (0, 0) else None))
            kbar_chain()
            for ic in range(IC):
                sim_block(ic)
                sbar_block(ic)
            # d = (1+s_ii)/(T+sbar) ~= (1+s_ii)*(1/T - sbar/T^2).  Carries a
            # 2^11 scale so d*2^11 ~ 1 keeps V8 = v*g*d~ in fp8 normal range;
            # the MM4 psum then holds 2^19*(out-bias) (W_out carries 2^8).
            num = statp.tile([P, IC], f32, tag="num")
            nc.vector.tensor_scalar(out=num[:], in0=dsim_all[:], scalar1=1.0,
                                    scalar2=1.0, op0=OP.mult, op1=OP.add)
            den = statp.tile([P, IC], f32, tag="den")
            nc.vector.tensor_scalar(out=den[:], in0=sb_all[:],
                                    scalar1=(-1.0 / (T * T)) * 2.0 ** 11,
                                    scalar2=(1.0 / T) * 2.0 ** 11,
                                    op0=OP.mult, op1=OP.add)
            dcol = statp.tile([P, IC], bf16, tag="dcol")
            nc.vector.tensor_tensor(out=dcol[:], in0=num[:], in1=den[:],
                                    op=OP.mult)
            diag_ap = diag_dr[:]
            diag_cols_ap = bass.AP(tensor=diag_ap.tensor, offset=diag_ap.offset,
                                   ap=[[1, P], [P, IC]])
            nc.sync.dma_start(out=diag_cols_ap, in_=dcol[:])
            # broadcast diag row to all partitions (SWDGE queue)
            scr_ap = diag_dr[:]
            bc_ap = bass.AP(tensor=scr_ap.tensor, offset=scr_ap.offset,
                            ap=[[0, P], [1, T]])
            nc.gpsimd.dma_start(out=dbcast[:], in_=bc_ap)

        # ---------------- Phase C: V8 = silu(xWv)*silu(xWg)*8 (fp8);
        #                  out = d * (W_out^T @ V) + b_out, window-major.
        with ExitStack() as cc:
            vstg = cc.enter_context(tc.tile_pool(name="vstg", bufs=6))
            ostg = cc.enter_context(tc.tile_pool(name="ostg", bufs=3))
            psV = cc.enter_context(tc.tile_pool(name="psV", bufs=2, space="PSUM"))
            psO = cc.enter_context(tc.tile_pool(name="psO", bufs=2, space="PSUM"))

            def vg_half(w, oc, g):
                # g=0: v (bias bv), g=1: gate (bias bg)
                ps = psV.tile([P, 2, FT], f32, tag="vgps")
                for h in range(2):
                    for c in range(DC // 2):
                        nc.tensor.matmul(ps[:, h, :],
                                         lhsT=wh8_sb[:, 2 * c:2 * c + 2,
                                                     ts(g * OC + oc, P)],
                                         rhs=x8_sb[:, 2 * c:2 * c + 2,
                                                   ts(2 * w + h, FT)],
                                         start=(c == 0), stop=(c == DC // 2 - 1),
                                         perf_mode=mybir.MatmulPerfMode.DoubleRow)
                s = vstg.tile([P, 2, FT], bf16, tag="svg")
                bias = (bv_sb if g == 0 else bg_sb)
                nc.scalar.activation(out=s[:], in_=ps[:], func=AF.Silu,
                                     bias=bias[:, oc:oc + 1], scale=2.0 ** -8)
                return s

            def vg_block(w, oc):
                sv = vg_half(w, oc, 0)
                sg = vg_half(w, oc, 1)
                # V8 = sv*sg*(d*2^11) -> fp8.  Step 1 all-bf16 on DVE
                # (2x mode); step 2 folds the diag in and quantizes,
                # alternating Pool/DVE so neither engine paces the chain.
                vt = vstg.tile([P, 2, FT], bf16, tag="vt")
                nc.vector.tensor_tensor(out=vt[:], in0=sv[:], in1=sg[:],
                                        op=OP.mult)
                eng = nc.gpsimd if oc % 2 == 0 else nc.vector
                eng.tensor_tensor(out=V8_sb[:, oc, w], in0=vt[:],
                                  in1=dbcast[:, w], op=OP.mult)

            def mm4_block(w, ncb, on_act):
                ops = psO.tile([P, 2, FT], f32, tag="ops")
                for h in range(2):
                    for c in range(OC // 2):
                        nc.tensor.matmul(ops[:, h, :],
                                         lhsT=wo8_sb[:, 2 * c:2 * c + 2,
                                                     ts(ncb, P)],
                                         rhs=V8_sb[:, 2 * c:2 * c + 2, w, h],
                                         start=(c == 0), stop=(c == OC // 2 - 1),
                                         perf_mode=mybir.MatmulPerfMode.DoubleRow)
                od = ostg.tile([P, 2, FT], f32, tag="od")
                # out = 2^-19 * psum + b_out: Identity activation is the
                # native scale+bias form; w0 rides ACT (interleaves with w1
                # silus), w1 rides DVE (emitted last, nothing queues behind)
                if on_act:
                    nc.scalar.activation(out=od[:], in_=ops[:],
                                         func=AF.Identity,
                                         bias=bo_sb[:, ncb:ncb + 1],
                                         scale=2.0 ** -19)
                else:
                    nc.vector.tensor_scalar(out=od[:], in0=ops[:],
                                            scalar1=2.0 ** -19,
                                            scalar2=bo_sb[:, ncb:ncb + 1],
                                            op0=OP.mult, op1=OP.add)
                nc.sync.dma_start(out=outT[ts(ncb, P), ts(w, TW)], in_=od[:])

            for oc in range(OC):
                vg_block(0, oc)
            for i in range(NC_):       # interleave: MM4(w0) amid vg(w1)
                mm4_block(0, i, on_act=True)
                vg_block(1, i)
            for ncb in range(NC_):
                mm4_block(1, ncb, on_act=False)

    nc.compile()
    return nc


def _get_nc():
    global _compiled_nc
    if _compiled_nc is None:
        _compiled_nc = _build()
    return _compiled_nc


_runner = None


def _make_runner(nc=None):
    """Cached sharded executable over 8 cores (mirrors bass2jax.run_bass_via_pjrt
    multi-core path, but jit-cached so repeat calls skip re-tracing)."""
    import jax
    import numpy as _np
    from jax.experimental.shard_map import shard_map
    from jax.sharding import Mesh, NamedSharding, PartitionSpec
    from concourse import bass2jax, mybir

    if nc is None:
        nc = _get_nc()
    bass2jax.install_neuronx_cc_hook()
    assert nc.dbg_addr is None

    partition_name = nc.partition_id_tensor.name if nc.partition_id_tensor else None
    in_names, out_names, out_avals = [], [], []
    for alloc in nc.m.functions[0].allocations:
        if not isinstance(alloc, bass2jax.mybir.MemoryLocationSet):
            continue
        name = alloc.memorylocations[0].name
        if alloc.kind == "ExternalInput":
            if name != partition_name:
                in_names.append(name)
        elif alloc.kind == "ExternalOutput":
            out_names.append(name)
            out_avals.append(jax.core.ShapedArray(
                tuple(alloc.tensor_shape), mybir.dt.np(alloc.dtype)))
    n_params = len(in_names)
    all_names = in_names + out_names
    if partition_name is not None:
        all_names = all_names + [partition_name]

    def _body(*args):
        operands = list(args)
        if partition_name is not None:
            operands.append(bass2jax.partition_id_tensor())
        outs = bass2jax._bass_exec_p.bind(
            *operands,
            out_avals=tuple(out_avals),
            in_names=tuple(all_names),
            out_names=tuple(out_names),
            lowering_input_output_aliases=(),
            sim_require_finite=True,
            sim_require_nnan=True,
            nc=nc,
        )
        return tuple(outs)

    devices = jax.devices()[:B]
    mesh = Mesh(_np.asarray(devices), ("core",))
    spec = PartitionSpec("core")
    n_total = n_params + len(out_names)
    sharded = jax.jit(
        shard_map(_body, mesh=mesh, in_specs=(spec,) * n_total,
                  out_specs=(spec,) * len(out_names), check_rep=False),
        donate_argnums=tuple(range(n_params, n_total)), keep_unused=True)
    sharding = NamedSharding(mesh, spec)
    zeros_avals = [(tuple([B * a.shape[0]] + list(a.shape[1:])), a.dtype)
                   for a in out_avals]

    def make_zeros():
        return [jax.device_put(_np.zeros(s, d), sharding) for s, d in zeros_avals]

    def run(in_maps, device_inputs=None):
        if device_inputs is None:
            concat = [_np.concatenate([_np.asarray(m[n]) for m in in_maps], axis=0)
                      for n in in_names]
            device_inputs = [jax.device_put(a, sharding) for a in concat]
        outs = sharded(*device_inputs, *make_zeros())
        res = []
        for c in range(B):
            res.append({n: _np.asarray(outs[i]).reshape(B, *out_avals[i].shape)[c]
                        for i, n in enumerate(out_names)})
        return res, device_inputs, outs

    return run, in_names, sharding


def _get_runner():
    global _runner
    if _runner is None:
        _runner = _make_runner()
    return _runner


def _cols(v, n):
    return np.ascontiguousarray(np.asarray(v, dtype=np.float32).reshape(n, P).T)


def build_in_maps(x, W_hidden, b_hidden, W_qk, b_qk, gamma, beta, W_out, b_out):
    x = np.asarray(x, dtype=np.float32)
    gamma = np.asarray(gamma, dtype=np.float32)
    beta = np.asarray(beta, dtype=np.float32)
    from concourse import mybir
    f8np = mybir.dt.np(mybir.dt.float8e4)
    bh = np.asarray(b_hidden, dtype=np.float32)
    consts = np.stack([
        _cols(b_qk, OC), _cols(bh[:TFO], OC), _cols(bh[TFO:], OC),
        _cols(b_out, NC_),
        _cols(gamma[0] / 32.0, OC), _cols(beta[0] / 32.0, OC),
        _cols(gamma[1], OC), _cols(beta[1], OC),
        _cols(gamma[1], OC), _cols(beta[1] * float(T), OC),
    ], axis=1)
    shared = {
        "wqk8": (np.asarray(W_qk, dtype=np.float32) * 256.0).astype(f8np),
        "wh8": (np.asarray(W_hidden, dtype=np.float32) * 256.0).astype(f8np),
        "wo8": (np.asarray(W_out, dtype=np.float32) * 256.0).astype(f8np),
        "consts": np.ascontiguousarray(consts),
    }
    in_maps = []
    for b in range(B):
        xt = np.ascontiguousarray(x[b].T)
        in_maps.append(dict(shared, xT8=xt.astype(f8np)))
    return in_maps


def kernel(x, W_hidden, b_hidden, W_qk, b_qk, gamma, beta, W_out, b_out):
    in_maps = build_in_maps(x, W_hidden, b_hidden, W_qk, b_qk, gamma, beta,
                            W_out, b_out)
    run, _, _ = _get_runner()
    results, _, _ = run(in_maps)
    out = np.stack([results[b]["outT"] for b in range(B)])[:, None]
    return out
